# revision 1
# baseline (speedup 1.0000x reference)
"""Trainium2 Bass kernel for fused causal GQA attention block.

Reference computation (B=1, S=2048, H=4096, NH=32, NKV=8, HD=128):
    qkv = hs @ w_attn.T; rope(q), rope(k); causal GQA attention;
    out @ w_proj.T

Sharding (8 cores, tensor parallel): core i owns kv-group i = rows
[i*768, (i+1)*768) of w_attn (4 q heads + 1 k + 1 v head) and rows
[i*512, (i+1)*512) of w_proj.  Each core computes its 4 heads of
attention output transposed (feature-major); a seq-chunked AllGather
(4 x [512, 512] per core -> [4096, 512]) assembles attnT on every core
pipelined against the next attention block, and each core computes its
512 output columns of the final projection per seq chunk.

All heavy matmuls run in float32r (tf32, full-rate fp32 PE path).
DMA triggers are spread across engine queues (sync / vector / scalar),
with gpsimd reserved for the collectives, and PSUM->SBUF copies run on
DVE to keep ACT free for the softmax exp.
"""

import sys

sys.path.insert(0, "/opt/trn_rl_repo")

import numpy as np

import concourse.bass as bass
import concourse.tile as tile
from concourse import bacc, mybir
from concourse.bass_utils import run_bass_kernel_spmd

F32 = mybir.dt.float32
F32R = mybir.dt.float32r

B, S, H = 1, 2048, 4096
NH, NKV, HD = 32, 8, 128
GROUP = NH // NKV  # 4
SCALE = 0.08838834764831845
NCORES = 8

M_SHARD = (GROUP + 2) * HD  # 768 rows of w_attn per core
P_SHARD = H // NCORES  # 512 rows of w_proj per core

KC = H // 128  # 32 contraction chunks of the model dim
NB = S // 512  # 4 seq blocks of 512
MT = M_SHARD // 128  # 6 row tiles of qkv_t
ST = S // 128  # 16 seq tiles of 128


def build_module() -> bass.Bass:
    nc = bacc.Bacc(
        "TRN2",
        target_bir_lowering=False,
        debug=False,
        num_devices=NCORES,
    )

    hs_t = nc.dram_tensor("hs_t", [H, S], F32R, kind="ExternalInput")
    wa_t = nc.dram_tensor("wa_t", [H, M_SHARD], F32R, kind="ExternalInput")
    wp_t = nc.dram_tensor("wp_t", [H, P_SHARD], F32R, kind="ExternalInput")
    cos_t = nc.dram_tensor("cos_t", [HD, S], F32R, kind="ExternalInput")
    sin_t = nc.dram_tensor("sin_t", [HD, S], F32R, kind="ExternalInput")
    rot_t = nc.dram_tensor("rot_t", [HD, HD], F32R, kind="ExternalInput")
    mask_sl = nc.dram_tensor("mask_sl", [128, 1024], F32R, kind="ExternalInput")
    ones_in = nc.dram_tensor("ones_in", [128, 128], F32R, kind="ExternalInput")
    ident_in = nc.dram_tensor("ident_in", [128, 128], F32R, kind="ExternalInput")
    y_out = nc.dram_tensor("y", [S, P_SHARD], F32, kind="ExternalOutput")

    # per-seq-chunk collective buffers
    ag_ins = [
        nc.dram_tensor(f"ag_in{i}", [GROUP * HD, 512], F32R, kind="Internal")
        for i in range(NB)
    ]
    ag_outs = [
        nc.dram_tensor(
            f"ag_out{i}", [H, 512], F32R, kind="Internal", addr_space="Shared"
        )
        for i in range(NB)
    ]

    # DRAM views with 128-partition tiling of the contraction axis
    hs_v = hs_t[:].rearrange("(ko p) n -> p ko n", p=128)  # [128, 32, 2048]
    wa_v = wa_t[:].rearrange("(ko p) m -> p ko m", p=128)  # [128, 32, 768]
    wp_v = wp_t[:].rearrange("(ko p) m -> p ko m", p=128)  # [128, 32, 512]
    ag_vs = [a[:].rearrange("(ko p) n -> p ko n", p=128) for a in ag_outs]

    with tile.TileContext(nc) as tc:
        # ---------- persistent pools ----------
        qkv_pool = tc.alloc_tile_pool(name="qkv", bufs=1)
        const_pool = tc.alloc_tile_pool(name="consts", bufs=1)

        qkv_sb = qkv_pool.tile([128, MT, S], F32R)  # 48KB/part

        ones_sb = const_pool.tile([128, 128], F32R)
        ident_sb = const_pool.tile([128, 128], F32R)
        rot_sb = const_pool.tile([128, HD], F32R)
        mask_sb = const_pool.tile([128, 1024], F32R)
        nc.scalar.dma_start(out=ones_sb, in_=ones_in[:])
        nc.scalar.dma_start(out=ident_sb, in_=ident_in[:])
        nc.scalar.dma_start(out=rot_sb, in_=rot_t[:])
        nc.scalar.dma_start(out=mask_sb, in_=mask_sl[:])

        # ---------- phase A: qkv_t = wa_shard @ hs.T ----------
        with (
            tc.tile_pool(name="wa", bufs=1) as wa_pool,
            tc.tile_pool(name="hs", bufs=2) as hs_pool,
            tc.tile_pool(name="psA", bufs=1, space="PSUM") as psA,
        ):
            wa_sb = wa_pool.tile([128, KC, M_SHARD], F32R)  # 96KB/part
            # wa on the vector queue so it overlaps the hs stream on sync
            for si, kk in enumerate(range(0, KC, 8)):
                eng = nc.scalar if si % 2 == 0 else nc.sync
                eng.dma_start(
                    out=wa_sb[:, kk : kk + 8, :], in_=wa_v[:, kk : kk + 8, :]
                )
            KSLAB = 8
            for nb in range(NB):
                psums = [
                    psA.tile([128, 512], F32, tag=f"ps{m}", name=f"psA{m}")
                    for m in range(MT)
                ]
                for ks in range(0, KC, KSLAB):
                    hs_slab = hs_pool.tile(
                        [128, KSLAB, 512], F32R, name="hs_slab"
                    )  # 16KB/part
                    eng = nc.sync if (nb * 4 + ks // KSLAB) % 2 == 0 else nc.scalar
                    eng.dma_start(
                        out=hs_slab,
                        in_=hs_v[:, ks : ks + KSLAB, nb * 512 : (nb + 1) * 512],
                    )
                    for k in range(ks, ks + KSLAB):
                        for m in range(MT):
                            nc.tensor.matmul(
                                psums[m],
                                lhsT=wa_sb[:, k, m * 128 : (m + 1) * 128],
                                rhs=hs_slab[:, k - ks, :],
                                start=(k == 0),
                                stop=(k == KC - 1),
                            )
                for m in range(MT):
                    nc.vector.tensor_copy(
                        out=qkv_sb[:, m, nb * 512 : (nb + 1) * 512], in_=psums[m]
                    )

        # ---------- phase B+C: rope, attention, chunked AG, c_proj ----------
        with (
            tc.tile_pool(name="wp", bufs=1) as wp_pool,
            tc.tile_pool(name="vnat", bufs=1) as vnat_pool,
        ):
            # w_proj shard: DMA overlaps rope/attention compute
            wp_sb = wp_pool.tile([128, KC, P_SHARD], F32R)  # 64KB/part
            for kk in range(0, KC, 8):
                nc.scalar.dma_start(
                    out=wp_sb[:, kk : kk + 8, :], in_=wp_v[:, kk : kk + 8, :]
                )

            v_nat = vnat_pool.tile([128, ST, HD], F32R)  # 8KB/part

            with (
                tc.tile_pool(name="rope", bufs=2) as rope_pool,
                tc.tile_pool(name="psR", bufs=2, space="PSUM") as psR,
            ):
                cos_sb = rope_pool.tile([128, S], F32R, tag="cos")
                sin_sb = rope_pool.tile([128, S], F32R, tag="sin")
                nc.sync.dma_start(out=cos_sb, in_=cos_t[:])
                nc.sync.dma_start(out=sin_sb, in_=sin_t[:])

                # rope on q0..q3 and k (tiles 0..4 of qkv_sb), in place
                for t in range(GROUP + 1):
                    x = qkv_sb[:, t, :]
                    for blk in range(NB):
                        rp = psR.tile([128, 512], F32, name="rp")
                        nc.tensor.matmul(
                            rp,
                            lhsT=rot_sb[:],
                            rhs=x[:, blk * 512 : (blk + 1) * 512],
                            start=True,
                            stop=True,
                        )
                        rs = rope_pool.tile([128, 512], F32R, tag="rs", name="rs")
                        nc.vector.tensor_mul(
                            rs, rp, sin_sb[:, blk * 512 : (blk + 1) * 512]
                        )
                        nc.vector.tensor_mul(
                            x[:, blk * 512 : (blk + 1) * 512],
                            x[:, blk * 512 : (blk + 1) * 512],
                            cos_sb[:, blk * 512 : (blk + 1) * 512],
                        )
                        nc.vector.tensor_add(
                            x[:, blk * 512 : (blk + 1) * 512],
                            x[:, blk * 512 : (blk + 1) * 512],
                            rs,
                        )

                # v natural layout: 16 PE transposes of vT chunks
                for j in range(ST):
                    tp = psR.tile([128, 128], F32R, tag="tp", name="tp")
                    nc.tensor.transpose(
                        tp,
                        qkv_sb[:, GROUP + 1, j * 128 : (j + 1) * 128],
                        ident_sb[:],
                    )
                    nc.vector.tensor_copy(out=v_nat[:, j, :], in_=tp)

            with (
                tc.tile_pool(name="pt", bufs=3) as pt_pool,
                tc.tile_pool(name="attn", bufs=2) as attn_pool,
                tc.tile_pool(name="agl", bufs=2) as agl_pool,
                tc.tile_pool(name="ysb", bufs=2) as y_pool,
                tc.tile_pool(name="psS", bufs=2, space="PSUM") as psS,
                tc.tile_pool(name="psL", bufs=2, space="PSUM") as psL,
                tc.tile_pool(name="psO", bufs=2, space="PSUM") as psO,
                tc.tile_pool(name="psC", bufs=2, space="PSUM") as psC,
            ):
                kT = qkv_sb[:, GROUP, :]
                for iq in range(NB):
                    njb = 4 * iq + 4
                    for h in range(GROUP):
                        qs = qkv_sb[:, h, iq * 512 : (iq + 1) * 512]
                        l_ps = psL.tile([128, 512], F32, name="l_ps")
                        o_ps = psO.tile([128, 512], F32, name="o_ps")
                        for j in range(njb):
                            st = psS.tile([128, 512], F32, name="st")
                            nc.tensor.matmul(
                                st,
                                lhsT=kT[:, j * 128 : (j + 1) * 128],
                                rhs=qs,
                                start=True,
                                stop=True,
                            )
                            pt = pt_pool.tile([128, 512], F32R, name="pt")
                            nc.scalar.activation(
                                out=pt,
                                in_=st,
                                func=mybir.ActivationFunctionType.Exp,
                                scale=SCALE,
                            )
                            off = j * 128 - iq * 512
                            if off >= 0:  # diagonal chunk: causal 0/1 mask
                                nc.vector.tensor_mul(
                                    pt, pt, mask_sb[:, 512 - off : 1024 - off]
                                )
                            nc.tensor.matmul(
                                l_ps,
                                lhsT=ones_sb[:],
                                rhs=pt[:],
                                start=(j == 0),
                                stop=(j == njb - 1),
                            )
                            nc.tensor.matmul(
                                o_ps,
                                lhsT=v_nat[:, j, :],
                                rhs=pt[:],
                                start=(j == 0),
                                stop=(j == njb - 1),
                            )
                        linv = attn_pool.tile(
                            [128, 512], F32, tag="linv", name="linv"
                        )
                        nc.vector.reciprocal(linv, l_ps)
                        at = attn_pool.tile([128, 512], F32R, tag="at", name="at")
                        nc.vector.tensor_mul(at, o_ps, linv)
                        nc.sync.dma_start(
                            out=ag_ins[iq][h * 128 : (h + 1) * 128, :], in_=at
                        )

                    # ---- seq-chunked AllGather (overlaps next iq's compute)
                    nc.gpsimd.collective_compute(
                        "AllGather",
                        mybir.AluOpType.bypass,
                        replica_groups=[list(range(NCORES))],
                        ins=[ag_ins[iq][:]],
                        outs=[ag_outs[iq][:]],
                    )

                    # ---- c_proj for this seq chunk: 4 row-tiles of 128
                    for sub in range(4):
                        mt = iq * 4 + sub
                        lh = agl_pool.tile([128, KC, 128], F32R, name="lh")
                        nc.sync.dma_start(
                            out=lh,
                            in_=ag_vs[iq][:, :, sub * 128 : (sub + 1) * 128],
                        )
                        yp = psC.tile([128, 512], F32, name="yp")
                        for k in range(KC):
                            nc.tensor.matmul(
                                yp,
                                lhsT=lh[:, k, :],
                                rhs=wp_sb[:, k, :],
                                start=(k == 0),
                                stop=(k == KC - 1),
                            )
                        ysb = y_pool.tile([128, P_SHARD], F32, name="ysb")
                        nc.vector.tensor_copy(out=ysb, in_=yp)
                        nc.sync.dma_start(
                            out=y_out[mt * 128 : (mt + 1) * 128, :], in_=ysb
                        )

        const_pool.release()
        qkv_pool.release()

    nc.compile()
    return nc


_CACHED = {}


def _get_module():
    if "nc" not in _CACHED:
        _CACHED["nc"] = build_module()
    return _CACHED["nc"]


def make_in_maps(hidden_states, w_attn, w_proj, rope_cos, rope_sin):
    hidden_states = np.asarray(hidden_states, dtype=np.float32)
    w_attn = np.asarray(w_attn, dtype=np.float32)
    w_proj = np.asarray(w_proj, dtype=np.float32)
    rope_cos = np.asarray(rope_cos, dtype=np.float32)
    rope_sin = np.asarray(rope_sin, dtype=np.float32)

    hs_t = np.ascontiguousarray(hidden_states.reshape(S, H).T)
    cos_t = np.ascontiguousarray(rope_cos.T)
    sin_t = np.ascontiguousarray(rope_sin.T)

    # rotate-half as a matmul: rot(x) = R @ x for x in [HD, S] layout,
    # rot_t = R.T so that lhsT.T @ x = R @ x
    rot_t = np.zeros((HD, HD), dtype=np.float32)
    half = HD // 2
    rot_t[half + np.arange(half), np.arange(half)] = -1.0
    rot_t[np.arange(half), half + np.arange(half)] = 1.0

    # causal staircase: mask_sl[r, c] = 1 iff c >= r + 512
    rr, cc = np.meshgrid(np.arange(128), np.arange(1024), indexing="ij")
    mask_sl = (cc >= rr + 512).astype(np.float32)

    ones = np.ones((128, 128), dtype=np.float32)
    ident = np.eye(128, dtype=np.float32)

    in_maps = []
    for i in range(NCORES):
        wa_sh = w_attn[i * M_SHARD : (i + 1) * M_SHARD, :]
        wp_sh = w_proj[i * P_SHARD : (i + 1) * P_SHARD, :]
        in_maps.append(
            {
                "hs_t": hs_t,
                "wa_t": np.ascontiguousarray(wa_sh.T),
                "wp_t": np.ascontiguousarray(wp_sh.T),
                "cos_t": cos_t,
                "sin_t": sin_t,
                "rot_t": rot_t,
                "mask_sl": mask_sl,
                "ones_in": ones,
                "ident_in": ident,
            }
        )
    return in_maps


def kernel(hidden_states, w_attn, w_proj, rope_cos, rope_sin, **_unused):
    nc = _get_module()
    in_maps = make_in_maps(hidden_states, w_attn, w_proj, rope_cos, rope_sin)
    res = run_bass_kernel_spmd(nc, in_maps, core_ids=list(range(NCORES)))

    out = np.empty((S, H), dtype=np.float32)
    for i in range(NCORES):
        out[:, i * P_SHARD : (i + 1) * P_SHARD] = res.results[i]["y"]
    return out.reshape(B, S, H)



# revision 5
# speedup vs baseline: 1.4957x; 1.4957x over previous
"""Trainium2 Bass kernel for fused causal GQA attention block.

Reference computation (B=1, S=2048, H=4096, NH=32, NKV=8, HD=128):
    qkv = hs @ w_attn.T; rope(q), rope(k); causal GQA attention;
    out @ w_proj.T

Sharding (8 cores, tensor parallel): core i owns kv-group i = rows
[i*768, (i+1)*768) of w_attn (4 q heads + 1 k + 1 v head) and rows
[i*512, (i+1)*512) of w_proj.

All heavy compute runs in bf16 (fp32 PSUM accumulation): full-rate PE
with fast weight load, half the DMA/SBUF/collective traffic of fp32.

Schedule: for each 512-seq block nb: QKV GEMM (2 passes of 3 qkv row
tiles over streamed hs slabs) -> rope(q,k) on DVE + V transpose via
XBAR DMA -> attention chunk nb (4 q-blocks of 128, all 4 heads fused
into the 512-wide free dim; causal mask added in PSUM; exp on ACT
pipelined 2 deep against the score matmuls) -> AllGather of the
block's attention output (bf16) fired immediately so all 4 collectives
hide under compute. Final c_proj consumes gathered chunks.
"""

import sys

sys.path.insert(0, "/opt/trn_rl_repo")

import ml_dtypes
import numpy as np

import concourse.bass as bass
import concourse.tile as tile
from concourse import bacc, mybir
from concourse.bass_utils import run_bass_kernel_spmd

F32 = mybir.dt.float32
BF16 = mybir.dt.bfloat16
BF16NP = ml_dtypes.bfloat16

B, S, H = 1, 2048, 4096
NH, NKV, HD = 32, 8, 128
GROUP = NH // NKV  # 4
SCALE = 0.08838834764831845
NCORES = 8

M_SHARD = (GROUP + 2) * HD  # 768 rows of w_attn per core
P_SHARD = H // NCORES  # 512 rows of w_proj per core

KC = H // 128  # 32 contraction chunks of the model dim
NB = S // 512  # 4 seq blocks of 512
MT = M_SHARD // 128  # 6 row tiles of qkv_t
QT = S // 128  # 16 q blocks of 128
MASKBIG = -600.0  # additive causal mask (-600 * SCALE ~ -53 before exp)


def build_module() -> bass.Bass:
    nc = bacc.Bacc(
        "TRN2",
        target_bir_lowering=False,
        debug=False,
        num_devices=NCORES,
    )

    hs_t = nc.dram_tensor("hs_t", [H, S], BF16, kind="ExternalInput")
    wa_t = nc.dram_tensor("wa_t", [H, M_SHARD], BF16, kind="ExternalInput")
    wp_t = nc.dram_tensor("wp_t", [H, P_SHARD], BF16, kind="ExternalInput")
    cos_t = nc.dram_tensor("cos_t", [HD, S], BF16, kind="ExternalInput")
    sin_t = nc.dram_tensor("sin_t", [HD, S], BF16, kind="ExternalInput")
    rot_t = nc.dram_tensor("rot_t", [HD, HD], BF16, kind="ExternalInput")
    masks_in = nc.dram_tensor("masks_in", [128, 512], BF16, kind="ExternalInput")
    ones_in = nc.dram_tensor("ones_in", [128, 128], BF16, kind="ExternalInput")
    ident_in = nc.dram_tensor("ident_in", [128, 128], BF16, kind="ExternalInput")
    y_out = nc.dram_tensor("y", [S, P_SHARD], F32, kind="ExternalOutput")

    # per-seq-chunk collective buffers (bf16 halves the wire bytes)
    ag_ins = [
        nc.dram_tensor(f"ag_in{i}", [GROUP * HD, 512], BF16, kind="Internal")
        for i in range(NB)
    ]
    ag_outs = [
        nc.dram_tensor(
            f"ag_out{i}", [H, 512], BF16, kind="Internal", addr_space="Shared"
        )
        for i in range(NB)
    ]

    # DRAM views with 128-partition tiling of the contraction axis
    hs_v = hs_t[:].rearrange("(ko p) n -> p ko n", p=128)  # [128, 32, 2048]
    wa_v = wa_t[:].rearrange("(ko p) m -> p ko m", p=128)  # [128, 32, 768]
    wp_v = wp_t[:].rearrange("(ko p) m -> p ko m", p=128)  # [128, 32, 512]
    ag_rd = [a[:].rearrange("(ko p) n -> p ko n", p=128) for a in ag_outs]
    # write view: feature row h*128+d <- at[d (part), (h, qq)]
    ag_wr = [a[:].rearrange("(h d) s -> d h s", h=GROUP) for a in ag_ins]

    with tile.TileContext(nc) as tc:
        # ---------- persistent pools ----------
        qkv_pool = tc.alloc_tile_pool(name="qkv", bufs=1)
        w_pool = tc.alloc_tile_pool(name="w", bufs=1)
        const_pool = tc.alloc_tile_pool(name="consts", bufs=1)
        vnat_pool = tc.alloc_tile_pool(name="vnat", bufs=1)
        hs_pool = tc.alloc_tile_pool(name="hs", bufs=2)
        rope_pool = tc.alloc_tile_pool(name="rope", bufs=2)
        pt_pool = tc.alloc_tile_pool(name="pt", bufs=3)
        attn_pool = tc.alloc_tile_pool(name="attn", bufs=2)
        psST = tc.alloc_tile_pool(name="psST", bufs=3, space="PSUM")
        psLO = tc.alloc_tile_pool(name="psLO", bufs=1, space="PSUM")
        psA = tc.alloc_tile_pool(name="psA", bufs=1, space="PSUM")

        qkv_sb = qkv_pool.tile([128, MT, S], BF16)  # 24KB/part
        wa_sb = w_pool.tile([128, KC, M_SHARD], BF16)  # 48KB/part
        wp_sb = w_pool.tile([128, KC, P_SHARD], BF16)  # 32KB/part
        v_nat = vnat_pool.tile([128, QT, HD], BF16)  # 4KB/part

        ones_sb = const_pool.tile([128, 128], BF16, tag="ones")
        ident_sb = const_pool.tile([128, 128], BF16, tag="ident")
        rot_sb = const_pool.tile([128, HD], BF16, tag="rot")
        masks_sb = const_pool.tile([128, 512], BF16, tag="masks")
        cos_sb = const_pool.tile([128, S], BF16, tag="cos")
        sin_sb = const_pool.tile([128, S], BF16, tag="sin")

        # ---------- preloads ----------
        # wa first-group columns first so phase A can start ASAP
        for half in range(2):
            mc = slice(half * 384, (half + 1) * 384)
            for kk in range(0, KC, 8):
                nc.scalar.dma_start(
                    out=wa_sb[:, kk : kk + 8, mc], in_=wa_v[:, kk : kk + 8, mc]
                )
        nc.gpsimd.dma_start(out=cos_sb, in_=cos_t[:])
        nc.gpsimd.dma_start(out=sin_sb, in_=sin_t[:])
        nc.gpsimd.dma_start(out=ones_sb, in_=ones_in[:])
        nc.gpsimd.dma_start(out=ident_sb, in_=ident_in[:])
        nc.gpsimd.dma_start(out=rot_sb, in_=rot_t[:])
        nc.gpsimd.dma_start(out=masks_sb, in_=masks_in[:])
        for kk in range(0, KC, 8):
            nc.scalar.dma_start(
                out=wp_sb[:, kk : kk + 8, :], in_=wp_v[:, kk : kk + 8, :]
            )

        kT = qkv_sb[:, GROUP, :]

        for nb in range(NB):
            sl = slice(nb * 512, (nb + 1) * 512)

            # ---------- phase A: qkv_t[:, :, nb] = wa_shard @ hs[nb].T ----
            for g in range(2):
                ms = [3 * g, 3 * g + 1, 3 * g + 2]
                psums = [
                    psA.tile([128, 512], F32, tag=f"a{i}", name=f"psA{i}")
                    for i in range(3)
                ]
                for ks in range(0, KC, 8):
                    slab = hs_pool.tile([128, 8, 512], BF16, name="hs_slab")
                    nc.sync.dma_start(out=slab, in_=hs_v[:, ks : ks + 8, sl])
                    for k in range(ks, ks + 8):
                        for i, m in enumerate(ms):
                            nc.tensor.matmul(
                                psums[i],
                                lhsT=wa_sb[:, k, m * 128 : (m + 1) * 128],
                                rhs=slab[:, k - ks, :],
                                start=(k == 0),
                                stop=(k == KC - 1),
                            )
                for i, m in enumerate(ms):
                    nc.scalar.activation(
                        out=qkv_sb[:, m, sl],
                        in_=psums[i],
                        func=mybir.ActivationFunctionType.Copy,
                    )

            # ---------- rope on q0..q3 and k for this block, in place ----
            for t in range(GROUP + 1):
                x = qkv_sb[:, t, sl]
                rp = psST.tile([128, 512], F32, tag="st", name="rp")
                nc.tensor.matmul(rp, lhsT=rot_sb, rhs=x, start=True, stop=True)
                rs = rope_pool.tile([128, 512], BF16, name="rs")
                nc.vector.tensor_mul(rs, rp, sin_sb[:, sl])
                nc.vector.tensor_mul(x, x, cos_sb[:, sl])
                nc.vector.tensor_add(x, x, rs)

            # ---------- v natural layout via XBAR transpose DMA ----------
            for jj in range(4):
                j = nb * 4 + jj
                nc.scalar.dma_start_transpose(
                    out=v_nat[:, j, :],
                    in_=qkv_sb[:, GROUP + 1, j * 128 : (j + 1) * 128],
                )

            # ---------- attention chunk nb: q blocks of 128, heads fused -
            for qi in range(nb * 4, nb * 4 + 4):
                rhs_q = qkv_sb[:, 0:GROUP, qi * 128 : (qi + 1) * 128]
                njt = qi + 1
                l_ps = psLO.tile([128, 512], F32, tag="l", name="l_ps")
                o_ps = psLO.tile([128, 512], F32, tag="o", name="o_ps")

                def emit_lo(j, pt):
                    nc.tensor.matmul(
                        l_ps,
                        lhsT=ones_sb,
                        rhs=pt,
                        start=(j == 0),
                        stop=(j == njt - 1),
                    )
                    nc.tensor.matmul(
                        o_ps,
                        lhsT=v_nat[:, j, :],
                        rhs=pt,
                        start=(j == 0),
                        stop=(j == njt - 1),
                    )

                pend = []
                for j in range(njt):
                    st = psST.tile([128, 512], F32, tag="st", name="st")
                    diag = j == qi
                    nc.tensor.matmul(
                        st,
                        lhsT=kT[:, j * 128 : (j + 1) * 128],
                        rhs=rhs_q,
                        start=True,
                        stop=not diag,
                    )
                    if diag:  # add -600 above the in-block diagonal
                        nc.tensor.matmul(
                            st,
                            lhsT=ident_sb,
                            rhs=masks_sb,
                            start=False,
                            stop=True,
                        )
                    pt = pt_pool.tile([128, 512], BF16, name="pt")
                    nc.scalar.activation(
                        out=pt,
                        in_=st,
                        func=mybir.ActivationFunctionType.Exp,
                        scale=SCALE,
                    )
                    pend.append((j, pt))
                    if len(pend) > 2:
                        emit_lo(*pend.pop(0))
                for j, pt in pend:
                    emit_lo(j, pt)

                linv = attn_pool.tile([128, 512], F32, tag="linv", name="linv")
                nc.vector.reciprocal_approx_fast(linv, l_ps)
                at = attn_pool.tile([128, 512], BF16, tag="at", name="at")
                nc.vector.tensor_mul(at, o_ps, linv)
                qsub = qi % 4
                nc.sync.dma_start(
                    out=ag_wr[nb][:, :, qsub * 128 : (qsub + 1) * 128], in_=at
                )

            # ---- seq-chunked AllGather (overlaps all remaining compute)
            nc.gpsimd.collective_compute(
                "AllGather",
                mybir.AluOpType.bypass,
                replica_groups=[list(range(NCORES))],
                ins=[ag_ins[nb][:]],
                outs=[ag_outs[nb][:]],
            )

        psA.release()

        # ---------- c_proj: y[mt] = attnT[:, mt].T @ wp_shard ----------
        with (
            tc.tile_pool(name="lh", bufs=2) as lh_pool,
            tc.tile_pool(name="ysb", bufs=2) as y_pool,
            tc.tile_pool(name="psC", bufs=2, space="PSUM") as psC,
        ):
            for c in range(NB):
                for sub in range(4):
                    mt = c * 4 + sub
                    lh = lh_pool.tile([128, KC, 128], BF16, name="lh")
                    nc.sync.dma_start(
                        out=lh, in_=ag_rd[c][:, :, sub * 128 : (sub + 1) * 128]
                    )
                    yp = psC.tile([128, 512], F32, name="yp")
                    for k in range(KC):
                        nc.tensor.matmul(
                            yp,
                            lhsT=lh[:, k, :],
                            rhs=wp_sb[:, k, :],
                            start=(k == 0),
                            stop=(k == KC - 1),
                        )
                    ysb = y_pool.tile([128, P_SHARD], F32, name="ysb")
                    nc.scalar.activation(
                        out=ysb,
                        in_=yp,
                        func=mybir.ActivationFunctionType.Copy,
                    )
                    nc.gpsimd.dma_start(
                        out=y_out[mt * 128 : (mt + 1) * 128, :], in_=ysb
                    )

        for p in (
            psLO,
            psST,
            attn_pool,
            pt_pool,
            rope_pool,
            hs_pool,
            vnat_pool,
            const_pool,
            w_pool,
            qkv_pool,
        ):
            p.release()

    nc.compile()
    return nc


_CACHED = {}


def _get_module():
    if "nc" not in _CACHED:
        _CACHED["nc"] = build_module()
    return _CACHED["nc"]


def make_in_maps(hidden_states, w_attn, w_proj, rope_cos, rope_sin):
    hidden_states = np.asarray(hidden_states, dtype=np.float32)
    w_attn = np.asarray(w_attn, dtype=np.float32)
    w_proj = np.asarray(w_proj, dtype=np.float32)
    rope_cos = np.asarray(rope_cos, dtype=np.float32)
    rope_sin = np.asarray(rope_sin, dtype=np.float32)

    hs_t = np.ascontiguousarray(hidden_states.reshape(S, H).T).astype(BF16NP)
    cos_t = np.ascontiguousarray(rope_cos.T).astype(BF16NP)
    sin_t = np.ascontiguousarray(rope_sin.T).astype(BF16NP)

    # rotate-half as a matmul: rot(x) = R @ x for x in [HD, S] layout,
    # rot_t = R.T so that lhsT.T @ x = R @ x
    rot_t = np.zeros((HD, HD), dtype=np.float32)
    half = HD // 2
    rot_t[half + np.arange(half), np.arange(half)] = -1.0
    rot_t[np.arange(half), half + np.arange(half)] = 1.0
    rot_t = rot_t.astype(BF16NP)

    # additive causal mask for the diagonal 128x128 block, repeated for
    # the 4 fused heads: masks[k, h*128+qq] = MASKBIG iff qq < k
    kk_, qq_ = np.meshgrid(np.arange(128), np.arange(128), indexing="ij")
    m128 = np.where(qq_ < kk_, MASKBIG, 0.0).astype(np.float32)
    masks = np.tile(m128, (1, GROUP)).astype(BF16NP)

    ones = np.ones((128, 128), dtype=np.float32).astype(BF16NP)
    ident = np.eye(128, dtype=np.float32).astype(BF16NP)

    in_maps = []
    for i in range(NCORES):
        wa_sh = w_attn[i * M_SHARD : (i + 1) * M_SHARD, :]
        wp_sh = w_proj[i * P_SHARD : (i + 1) * P_SHARD, :]
        in_maps.append(
            {
                "hs_t": hs_t,
                "wa_t": np.ascontiguousarray(wa_sh.T).astype(BF16NP),
                "wp_t": np.ascontiguousarray(wp_sh.T).astype(BF16NP),
                "cos_t": cos_t,
                "sin_t": sin_t,
                "rot_t": rot_t,
                "masks_in": masks,
                "ones_in": ones,
                "ident_in": ident,
            }
        )
    return in_maps


def kernel(hidden_states, w_attn, w_proj, rope_cos, rope_sin, **_unused):
    nc = _get_module()
    in_maps = make_in_maps(hidden_states, w_attn, w_proj, rope_cos, rope_sin)
    res = run_bass_kernel_spmd(nc, in_maps, core_ids=list(range(NCORES)))

    out = np.empty((S, H), dtype=np.float32)
    for i in range(NCORES):
        out[:, i * P_SHARD : (i + 1) * P_SHARD] = res.results[i]["y"]
    return out.reshape(B, S, H)


# revision 6
# speedup vs baseline: 1.6051x; 1.0732x over previous
"""Trainium2 Bass kernel for fused causal GQA attention block.

Reference computation (B=1, S=2048, H=4096, NH=32, NKV=8, HD=128):
    qkv = hs @ w_attn.T; rope(q), rope(k); causal GQA attention;
    out @ w_proj.T

Sharding (8 cores, tensor parallel): core i owns kv-group i = rows
[i*768, (i+1)*768) of w_attn (4 q heads + 1 k + 1 v head) and rows
[i*512, (i+1)*512) of w_proj.

All heavy compute runs in bf16 (fp32 PSUM accumulation): full-rate PE
with fast weight load, half the DMA/SBUF/collective traffic of fp32.

Schedule: for each 512-seq block nb: QKV GEMM (2 passes of 3 qkv row
tiles over streamed hs slabs) -> rope(q,k) on DVE + V transpose via
XBAR DMA -> attention chunk nb (4 q-blocks of 128, all 4 heads fused
into the 512-wide free dim; causal mask added in PSUM; exp on ACT
pipelined 2 deep against the score matmuls) -> AllGather of the
block's attention output (bf16) fired immediately so all 4 collectives
hide under compute. Final c_proj consumes gathered chunks.
"""

import sys

sys.path.insert(0, "/opt/trn_rl_repo")

import ml_dtypes
import numpy as np

import concourse.bass as bass
import concourse.tile as tile
from concourse import bacc, mybir
from concourse.bass_utils import run_bass_kernel_spmd

F32 = mybir.dt.float32
BF16 = mybir.dt.bfloat16
BF16NP = ml_dtypes.bfloat16

B, S, H = 1, 2048, 4096
NH, NKV, HD = 32, 8, 128
GROUP = NH // NKV  # 4
SCALE = 0.08838834764831845
NCORES = 8

M_SHARD = (GROUP + 2) * HD  # 768 rows of w_attn per core
P_SHARD = H // NCORES  # 512 rows of w_proj per core

KC = H // 128  # 32 contraction chunks of the model dim
NB = S // 512  # 4 seq blocks of 512
MT = M_SHARD // 128  # 6 row tiles of qkv_t
QT = S // 128  # 16 q blocks of 128
MASKBIG = -600.0  # additive causal mask (-600 * SCALE ~ -53 before exp)


def build_module() -> bass.Bass:
    nc = bacc.Bacc(
        "TRN2",
        target_bir_lowering=False,
        debug=False,
        num_devices=NCORES,
    )

    hs_t = nc.dram_tensor("hs_t", [H, S], BF16, kind="ExternalInput")
    wa_t = nc.dram_tensor("wa_t", [H, M_SHARD], BF16, kind="ExternalInput")
    wp_t = nc.dram_tensor("wp_t", [H, P_SHARD], BF16, kind="ExternalInput")
    cos_t = nc.dram_tensor("cos_t", [HD, S], BF16, kind="ExternalInput")
    sin_t = nc.dram_tensor("sin_t", [HD, S], BF16, kind="ExternalInput")
    rot_t = nc.dram_tensor("rot_t", [HD, HD], BF16, kind="ExternalInput")
    masks_in = nc.dram_tensor("masks_in", [128, 512], BF16, kind="ExternalInput")
    ones_in = nc.dram_tensor("ones_in", [128, 128], BF16, kind="ExternalInput")
    ident_in = nc.dram_tensor("ident_in", [128, 128], BF16, kind="ExternalInput")
    y_out = nc.dram_tensor("y", [S, P_SHARD], F32, kind="ExternalOutput")

    # per-seq-chunk collective buffers (bf16 halves the wire bytes)
    ag_ins = [
        nc.dram_tensor(f"ag_in{i}", [GROUP * HD, 512], BF16, kind="Internal")
        for i in range(NB)
    ]
    ag_outs = [
        nc.dram_tensor(
            f"ag_out{i}", [H, 512], BF16, kind="Internal", addr_space="Shared"
        )
        for i in range(NB)
    ]

    # DRAM views with 128-partition tiling of the contraction axis
    hs_v = hs_t[:].rearrange("(ko p) n -> p ko n", p=128)  # [128, 32, 2048]
    wa_v = wa_t[:].rearrange("(ko p) m -> p ko m", p=128)  # [128, 32, 768]
    wp_v = wp_t[:].rearrange("(ko p) m -> p ko m", p=128)  # [128, 32, 512]
    ag_rd = [a[:].rearrange("(ko p) n -> p ko n", p=128) for a in ag_outs]
    # write view: feature row h*128+d <- at[d (part), (h, qq)]
    ag_wr = [a[:].rearrange("(h d) s -> d h s", h=GROUP) for a in ag_ins]

    with tile.TileContext(nc) as tc:
        # ---------- persistent pools ----------
        qkv_pool = tc.alloc_tile_pool(name="qkv", bufs=1)
        w_pool = tc.alloc_tile_pool(name="w", bufs=1)
        const_pool = tc.alloc_tile_pool(name="consts", bufs=1)
        vnat_pool = tc.alloc_tile_pool(name="vnat", bufs=1)
        rope_pool = tc.alloc_tile_pool(name="rope", bufs=2)
        pt_pool = tc.alloc_tile_pool(name="pt", bufs=4)
        attn_pool = tc.alloc_tile_pool(name="attn", bufs=2)
        psST = tc.alloc_tile_pool(name="psST", bufs=3, space="PSUM")
        psLO = tc.alloc_tile_pool(name="psLO", bufs=1, space="PSUM")
        hs_pool = tc.alloc_tile_pool(name="hs", bufs=2)
        psA = tc.alloc_tile_pool(name="psA", bufs=1, space="PSUM")

        qkv_sb = qkv_pool.tile([128, MT, S], BF16)  # 24KB/part
        wa_sb = w_pool.tile([128, KC, M_SHARD], BF16)  # 48KB/part
        v_nat = vnat_pool.tile([128, QT, HD], BF16)  # 4KB/part

        ones_sb = const_pool.tile([128, 128], BF16, tag="ones")
        ident_sb = const_pool.tile([128, 128], BF16, tag="ident")
        rot_sb = const_pool.tile([128, HD], BF16, tag="rot")
        masks_sb = const_pool.tile([128, 512], BF16, tag="masks")
        cos_sb = const_pool.tile([128, S], BF16, tag="cos")
        sin_sb = const_pool.tile([128, S], BF16, tag="sin")

        # ---------- preloads ----------
        # wa first-group columns first so phase A can start ASAP
        for half in range(2):
            mc = slice(half * 384, (half + 1) * 384)
            for kk in range(0, KC, 8):
                nc.scalar.dma_start(
                    out=wa_sb[:, kk : kk + 8, mc], in_=wa_v[:, kk : kk + 8, mc]
                )
        nc.gpsimd.dma_start(out=cos_sb, in_=cos_t[:])
        nc.gpsimd.dma_start(out=sin_sb, in_=sin_t[:])
        nc.gpsimd.dma_start(out=ones_sb, in_=ones_in[:])
        nc.gpsimd.dma_start(out=ident_sb, in_=ident_in[:])
        nc.gpsimd.dma_start(out=rot_sb, in_=rot_t[:])
        nc.gpsimd.dma_start(out=masks_sb, in_=masks_in[:])
        kT = qkv_sb[:, GROUP, :]

        for nb in range(NB):
            sl = slice(nb * 512, (nb + 1) * 512)

            # ---------- phase A: qkv_t[:, :, nb] = wa_shard @ hs[nb].T ----
            hs_nb = hs_pool.tile([128, KC, 512], BF16, name="hs_nb")
            for ks in range(0, KC, 8):
                nc.sync.dma_start(
                    out=hs_nb[:, ks : ks + 8, :], in_=hs_v[:, ks : ks + 8, sl]
                )
            for g in range(2):
                ms = [3 * g, 3 * g + 1, 3 * g + 2]
                psums = [
                    psA.tile([128, 512], F32, tag=f"a{i}", name=f"psA{i}")
                    for i in range(3)
                ]
                for k in range(KC):
                    for i, m in enumerate(ms):
                        nc.tensor.matmul(
                            psums[i],
                            lhsT=wa_sb[:, k, m * 128 : (m + 1) * 128],
                            rhs=hs_nb[:, k, :],
                            start=(k == 0),
                            stop=(k == KC - 1),
                        )
                for i, m in enumerate(ms):
                    nc.vector.tensor_copy(out=qkv_sb[:, m, sl], in_=psums[i])

            # ---------- rope on q0..q3 and k for this block, in place ----
            for t in range(GROUP + 1):
                x = qkv_sb[:, t, sl]
                rp = psST.tile([128, 512], F32, tag="st", name="rp")
                nc.tensor.matmul(rp, lhsT=rot_sb, rhs=x, start=True, stop=True)
                rs = rope_pool.tile([128, 512], BF16, name="rs")
                nc.vector.tensor_mul(rs, rp, sin_sb[:, sl])
                nc.vector.tensor_mul(x, x, cos_sb[:, sl])
                nc.vector.tensor_add(x, x, rs)

            # ---------- v natural layout via XBAR transpose DMA ----------
            for jj in range(4):
                j = nb * 4 + jj
                nc.scalar.dma_start_transpose(
                    out=v_nat[:, j, :],
                    in_=qkv_sb[:, GROUP + 1, j * 128 : (j + 1) * 128],
                )

            # ---------- attention chunk nb: q blocks of 128, heads fused -
            for qi in range(nb * 4, nb * 4 + 4):
                rhs_q = qkv_sb[:, 0:GROUP, qi * 128 : (qi + 1) * 128]
                njt = qi + 1
                l_ps = psLO.tile([128, 512], F32, tag="l", name="l_ps")
                o_ps = psLO.tile([128, 512], F32, tag="o", name="o_ps")

                def emit_lo(j, pt):
                    nc.tensor.matmul(
                        l_ps,
                        lhsT=ones_sb,
                        rhs=pt,
                        start=(j == 0),
                        stop=(j == njt - 1),
                    )
                    nc.tensor.matmul(
                        o_ps,
                        lhsT=v_nat[:, j, :],
                        rhs=pt,
                        start=(j == 0),
                        stop=(j == njt - 1),
                    )

                pend = []
                for j in range(njt):
                    st = psST.tile([128, 512], F32, tag="st", name="st")
                    diag = j == qi
                    nc.tensor.matmul(
                        st,
                        lhsT=kT[:, j * 128 : (j + 1) * 128],
                        rhs=rhs_q,
                        start=True,
                        stop=not diag,
                    )
                    if diag:  # add -600 above the in-block diagonal
                        nc.tensor.matmul(
                            st,
                            lhsT=ident_sb,
                            rhs=masks_sb,
                            start=False,
                            stop=True,
                        )
                    pt = pt_pool.tile([128, 512], BF16, name="pt")
                    nc.scalar.activation(
                        out=pt,
                        in_=st,
                        func=mybir.ActivationFunctionType.Exp,
                        scale=SCALE,
                    )
                    pend.append((j, pt))
                    if len(pend) > 2:
                        emit_lo(*pend.pop(0))
                for j, pt in pend:
                    emit_lo(j, pt)

                linv = attn_pool.tile([128, 512], F32, tag="linv", name="linv")
                nc.vector.reciprocal_approx_fast(linv, l_ps)
                at = attn_pool.tile([128, 512], BF16, tag="at", name="at")
                nc.vector.tensor_mul(at, o_ps, linv)
                qsub = qi % 4
                nc.gpsimd.dma_start(
                    out=ag_wr[nb][:, :, qsub * 128 : (qsub + 1) * 128], in_=at
                )

            # ---- seq-chunked AllGather (overlaps all remaining compute)
            nc.gpsimd.collective_compute(
                "AllGather",
                mybir.AluOpType.bypass,
                replica_groups=[list(range(NCORES))],
                ins=[ag_ins[nb][:]],
                outs=[ag_outs[nb][:]],
            )

        psA.release()
        hs_pool.release()

        # ---------- c_proj: y[mt] = attnT[:, mt].T @ wp_shard ----------
        with (
            tc.tile_pool(name="wp", bufs=1) as wp_pool,
            tc.tile_pool(name="lh", bufs=2) as lh_pool,
            tc.tile_pool(name="ysb", bufs=2) as y_pool,
            tc.tile_pool(name="psC", bufs=2, space="PSUM") as psC,
        ):
            wp_sb = wp_pool.tile([128, KC, P_SHARD], BF16)  # 32KB/part
            for kk in range(0, KC, 8):
                nc.sync.dma_start(
                    out=wp_sb[:, kk : kk + 8, :], in_=wp_v[:, kk : kk + 8, :]
                )
            for c in range(NB):
                for sub in range(4):
                    mt = c * 4 + sub
                    lh = lh_pool.tile([128, KC, 128], BF16, name="lh")
                    nc.sync.dma_start(
                        out=lh, in_=ag_rd[c][:, :, sub * 128 : (sub + 1) * 128]
                    )
                    yp = psC.tile([128, 512], F32, name="yp")
                    for k in range(KC):
                        nc.tensor.matmul(
                            yp,
                            lhsT=lh[:, k, :],
                            rhs=wp_sb[:, k, :],
                            start=(k == 0),
                            stop=(k == KC - 1),
                        )
                    ysb = y_pool.tile([128, P_SHARD], F32, name="ysb")
                    nc.scalar.activation(
                        out=ysb,
                        in_=yp,
                        func=mybir.ActivationFunctionType.Copy,
                    )
                    nc.gpsimd.dma_start(
                        out=y_out[mt * 128 : (mt + 1) * 128, :], in_=ysb
                    )

        for p in (
            psLO,
            psST,
            attn_pool,
            pt_pool,
            rope_pool,
            vnat_pool,
            const_pool,
            w_pool,
            qkv_pool,
        ):
            p.release()

    nc.compile()
    return nc


_CACHED = {}


def _get_module():
    if "nc" not in _CACHED:
        _CACHED["nc"] = build_module()
    return _CACHED["nc"]


def make_in_maps(hidden_states, w_attn, w_proj, rope_cos, rope_sin):
    hidden_states = np.asarray(hidden_states, dtype=np.float32)
    w_attn = np.asarray(w_attn, dtype=np.float32)
    w_proj = np.asarray(w_proj, dtype=np.float32)
    rope_cos = np.asarray(rope_cos, dtype=np.float32)
    rope_sin = np.asarray(rope_sin, dtype=np.float32)

    hs_t = np.ascontiguousarray(hidden_states.reshape(S, H).T).astype(BF16NP)
    cos_t = np.ascontiguousarray(rope_cos.T).astype(BF16NP)
    sin_t = np.ascontiguousarray(rope_sin.T).astype(BF16NP)

    # rotate-half as a matmul: rot(x) = R @ x for x in [HD, S] layout,
    # rot_t = R.T so that lhsT.T @ x = R @ x
    rot_t = np.zeros((HD, HD), dtype=np.float32)
    half = HD // 2
    rot_t[half + np.arange(half), np.arange(half)] = -1.0
    rot_t[np.arange(half), half + np.arange(half)] = 1.0
    rot_t = rot_t.astype(BF16NP)

    # additive causal mask for the diagonal 128x128 block, repeated for
    # the 4 fused heads: masks[k, h*128+qq] = MASKBIG iff qq < k
    kk_, qq_ = np.meshgrid(np.arange(128), np.arange(128), indexing="ij")
    m128 = np.where(qq_ < kk_, MASKBIG, 0.0).astype(np.float32)
    masks = np.tile(m128, (1, GROUP)).astype(BF16NP)

    ones = np.ones((128, 128), dtype=np.float32).astype(BF16NP)
    ident = np.eye(128, dtype=np.float32).astype(BF16NP)

    in_maps = []
    for i in range(NCORES):
        wa_sh = w_attn[i * M_SHARD : (i + 1) * M_SHARD, :]
        wp_sh = w_proj[i * P_SHARD : (i + 1) * P_SHARD, :]
        in_maps.append(
            {
                "hs_t": hs_t,
                "wa_t": np.ascontiguousarray(wa_sh.T).astype(BF16NP),
                "wp_t": np.ascontiguousarray(wp_sh.T).astype(BF16NP),
                "cos_t": cos_t,
                "sin_t": sin_t,
                "rot_t": rot_t,
                "masks_in": masks,
                "ones_in": ones,
                "ident_in": ident,
            }
        )
    return in_maps


def kernel(hidden_states, w_attn, w_proj, rope_cos, rope_sin, **_unused):
    nc = _get_module()
    in_maps = make_in_maps(hidden_states, w_attn, w_proj, rope_cos, rope_sin)
    res = run_bass_kernel_spmd(nc, in_maps, core_ids=list(range(NCORES)))

    out = np.empty((S, H), dtype=np.float32)
    for i in range(NCORES):
        out[:, i * P_SHARD : (i + 1) * P_SHARD] = res.results[i]["y"]
    return out.reshape(B, S, H)


# revision 7
# speedup vs baseline: 1.6456x; 1.0253x over previous
"""Trainium2 Bass kernel for fused causal GQA attention block.

Reference computation (B=1, S=2048, H=4096, NH=32, NKV=8, HD=128):
    qkv = hs @ w_attn.T; rope(q), rope(k); causal GQA attention;
    out @ w_proj.T

Sharding (8 cores, tensor parallel): core i owns kv-group i = rows
[i*768, (i+1)*768) of w_attn (4 q heads + 1 k + 1 v head) and rows
[i*512, (i+1)*512) of w_proj.

All heavy compute runs in bf16 (fp32 PSUM accumulation): full-rate PE
with fast weight load, half the DMA/SBUF/collective traffic of fp32.

Schedule: for each 512-seq block nb: QKV GEMM (2 passes of 3 qkv row
tiles over streamed hs slabs) -> rope(q,k) on DVE + V transpose via
XBAR DMA -> attention chunk nb (4 q-blocks of 128, all 4 heads fused
into the 512-wide free dim; causal mask added in PSUM; exp on ACT
pipelined 2 deep against the score matmuls) -> AllGather of the
block's attention output (bf16) fired immediately so all 4 collectives
hide under compute. Final c_proj consumes gathered chunks.
"""

import sys

sys.path.insert(0, "/opt/trn_rl_repo")

import ml_dtypes
import numpy as np

import concourse.bass as bass
import concourse.tile as tile
from concourse import bacc, mybir
from concourse.bass_utils import run_bass_kernel_spmd

F32 = mybir.dt.float32
BF16 = mybir.dt.bfloat16
BF16NP = ml_dtypes.bfloat16

B, S, H = 1, 2048, 4096
NH, NKV, HD = 32, 8, 128
GROUP = NH // NKV  # 4
SCALE = 0.08838834764831845
NCORES = 8

M_SHARD = (GROUP + 2) * HD  # 768 rows of w_attn per core
P_SHARD = H // NCORES  # 512 rows of w_proj per core

KC = H // 128  # 32 contraction chunks of the model dim
NB = S // 512  # 4 seq blocks of 512
MT = M_SHARD // 128  # 6 row tiles of qkv_t
QT = S // 128  # 16 q blocks of 128
MASKBIG = -600.0  # additive causal mask (-600 * SCALE ~ -53 before exp)


def build_module() -> bass.Bass:
    nc = bacc.Bacc(
        "TRN2",
        target_bir_lowering=False,
        debug=False,
        num_devices=NCORES,
    )

    hs_t = nc.dram_tensor("hs_t", [H, S], BF16, kind="ExternalInput")
    wa_t = nc.dram_tensor("wa_t", [H, M_SHARD], BF16, kind="ExternalInput")
    wp_t = nc.dram_tensor("wp_t", [H, P_SHARD], BF16, kind="ExternalInput")
    cos_t = nc.dram_tensor("cos_t", [HD, S], BF16, kind="ExternalInput")
    sin_t = nc.dram_tensor("sin_t", [HD, S], BF16, kind="ExternalInput")
    rot_t = nc.dram_tensor("rot_t", [HD, HD], BF16, kind="ExternalInput")
    masks_in = nc.dram_tensor("masks_in", [128, 512], BF16, kind="ExternalInput")
    ones_in = nc.dram_tensor("ones_in", [128, 128], BF16, kind="ExternalInput")
    ident_in = nc.dram_tensor("ident_in", [128, 128], BF16, kind="ExternalInput")
    y_out = nc.dram_tensor("y", [S, P_SHARD], F32, kind="ExternalOutput")

    # per-seq-chunk collective buffers (bf16 halves the wire bytes)
    ag_ins = [
        nc.dram_tensor(f"ag_in{i}", [GROUP * HD, 512], BF16, kind="Internal")
        for i in range(NB)
    ]
    ag_outs = [
        nc.dram_tensor(
            f"ag_out{i}", [H, 512], BF16, kind="Internal", addr_space="Shared"
        )
        for i in range(NB)
    ]

    # DRAM views with 128-partition tiling of the contraction axis
    hs_v = hs_t[:].rearrange("(ko p) n -> p ko n", p=128)  # [128, 32, 2048]
    wa_v = wa_t[:].rearrange("(ko p) m -> p ko m", p=128)  # [128, 32, 768]
    wp_v = wp_t[:].rearrange("(ko p) m -> p ko m", p=128)  # [128, 32, 512]
    ag_rd = [a[:].rearrange("(ko p) n -> p ko n", p=128) for a in ag_outs]
    # write view: feature row h*128+d <- at[d (part), (h, qq)]
    ag_wr = [a[:].rearrange("(h d) s -> d h s", h=GROUP) for a in ag_ins]

    with tile.TileContext(nc) as tc:
        # ---------- persistent pools ----------
        qkv_pool = tc.alloc_tile_pool(name="qkv", bufs=1)
        w_pool = tc.alloc_tile_pool(name="w", bufs=1)
        const_pool = tc.alloc_tile_pool(name="consts", bufs=1)
        vnat_pool = tc.alloc_tile_pool(name="vnat", bufs=1)
        rope_pool = tc.alloc_tile_pool(name="rope", bufs=2)
        pt_pool = tc.alloc_tile_pool(name="pt", bufs=4)
        attn_pool = tc.alloc_tile_pool(name="attn", bufs=2)
        psST = tc.alloc_tile_pool(name="psST", bufs=3, space="PSUM")
        psLO = tc.alloc_tile_pool(name="psLO", bufs=1, space="PSUM")
        hs_pool = tc.alloc_tile_pool(name="hs", bufs=2)
        psA = tc.alloc_tile_pool(name="psA", bufs=1, space="PSUM")

        qkv_sb = qkv_pool.tile([128, MT, S], BF16)  # 24KB/part
        wa_sb = w_pool.tile([128, KC, M_SHARD], BF16)  # 48KB/part
        v_nat = vnat_pool.tile([128, QT, HD], BF16)  # 4KB/part

        ones_sb = const_pool.tile([128, 128], BF16, tag="ones")
        ident_sb = const_pool.tile([128, 128], BF16, tag="ident")
        rot_sb = const_pool.tile([128, HD], BF16, tag="rot")
        masks_sb = const_pool.tile([128, 512], BF16, tag="masks")
        cos_sb = const_pool.tile([128, S], BF16, tag="cos")
        sin_sb = const_pool.tile([128, S], BF16, tag="sin")

        # ---------- preloads ----------
        # wa first-group columns first so phase A can start ASAP
        for half in range(2):
            mc = slice(half * 384, (half + 1) * 384)
            bounds = [0, 2, 8, 16, 24, 32] if half == 0 else [0, 16, 32]
            for lo, hi in zip(bounds, bounds[1:]):
                nc.scalar.dma_start(
                    out=wa_sb[:, lo:hi, mc], in_=wa_v[:, lo:hi, mc]
                )
        nc.gpsimd.dma_start(out=cos_sb, in_=cos_t[:])
        nc.gpsimd.dma_start(out=sin_sb, in_=sin_t[:])
        nc.gpsimd.dma_start(out=ones_sb, in_=ones_in[:])
        nc.gpsimd.dma_start(out=ident_sb, in_=ident_in[:])
        nc.gpsimd.dma_start(out=rot_sb, in_=rot_t[:])
        nc.gpsimd.dma_start(out=masks_sb, in_=masks_in[:])
        kT = qkv_sb[:, GROUP, :]

        for nb in range(NB):
            sl = slice(nb * 512, (nb + 1) * 512)

            # ---------- phase A: qkv_t[:, :, nb] = wa_shard @ hs[nb].T ----
            hs_nb = hs_pool.tile([128, KC, 512], BF16, name="hs_nb")
            bounds = [0, 2, 8, 16, 24, 32] if nb == 0 else [0, 8, 16, 24, 32]
            for lo, hi in zip(bounds, bounds[1:]):
                nc.sync.dma_start(
                    out=hs_nb[:, lo:hi, :], in_=hs_v[:, lo:hi, sl]
                )
            for g in range(2):
                ms = [3 * g, 3 * g + 1, 3 * g + 2]
                psums = [
                    psA.tile([128, 512], F32, tag=f"a{i}", name=f"psA{i}")
                    for i in range(3)
                ]
                for k in range(KC):
                    for i, m in enumerate(ms):
                        nc.tensor.matmul(
                            psums[i],
                            lhsT=wa_sb[:, k, m * 128 : (m + 1) * 128],
                            rhs=hs_nb[:, k, :],
                            start=(k == 0),
                            stop=(k == KC - 1),
                        )
                for i, m in enumerate(ms):
                    nc.vector.tensor_copy(out=qkv_sb[:, m, sl], in_=psums[i])

            # ---------- rope on q0..q3 and k for this block, in place ----
            for t in range(GROUP + 1):
                x = qkv_sb[:, t, sl]
                rp = psST.tile([128, 512], F32, tag="st", name="rp")
                nc.tensor.matmul(rp, lhsT=rot_sb, rhs=x, start=True, stop=True)
                rs = rope_pool.tile([128, 512], BF16, name="rs")
                nc.vector.tensor_mul(rs, rp, sin_sb[:, sl])
                nc.vector.tensor_mul(x, x, cos_sb[:, sl])
                nc.vector.tensor_add(x, x, rs)

            # ---------- v natural layout via XBAR transpose DMA ----------
            for jj in range(4):
                j = nb * 4 + jj
                nc.sync.dma_start_transpose(
                    out=v_nat[:, j, :],
                    in_=qkv_sb[:, GROUP + 1, j * 128 : (j + 1) * 128],
                )

            # ---------- attention chunk nb: q blocks of 128, heads fused -
            for qi in range(nb * 4, nb * 4 + 4):
                rhs_q = qkv_sb[:, 0:GROUP, qi * 128 : (qi + 1) * 128]
                njt = qi + 1
                l_ps = psLO.tile([128, 512], F32, tag="l", name="l_ps")
                o_ps = psLO.tile([128, 512], F32, tag="o", name="o_ps")

                def emit_lo(j, pt):
                    nc.tensor.matmul(
                        l_ps,
                        lhsT=ones_sb,
                        rhs=pt,
                        start=(j == 0),
                        stop=(j == njt - 1),
                    )
                    nc.tensor.matmul(
                        o_ps,
                        lhsT=v_nat[:, j, :],
                        rhs=pt,
                        start=(j == 0),
                        stop=(j == njt - 1),
                    )

                pend = []
                for j in range(njt):
                    st = psST.tile([128, 512], F32, tag="st", name="st")
                    diag = j == qi
                    nc.tensor.matmul(
                        st,
                        lhsT=kT[:, j * 128 : (j + 1) * 128],
                        rhs=rhs_q,
                        start=True,
                        stop=not diag,
                    )
                    if diag:  # add -600 above the in-block diagonal
                        nc.tensor.matmul(
                            st,
                            lhsT=ident_sb,
                            rhs=masks_sb,
                            start=False,
                            stop=True,
                        )
                    pt = pt_pool.tile([128, 512], BF16, name="pt")
                    nc.scalar.activation(
                        out=pt,
                        in_=st,
                        func=mybir.ActivationFunctionType.Exp,
                        scale=SCALE,
                    )
                    pend.append((j, pt))
                    if len(pend) > 2:
                        emit_lo(*pend.pop(0))
                for j, pt in pend:
                    emit_lo(j, pt)

                linv = attn_pool.tile([128, 512], F32, tag="linv", name="linv")
                nc.vector.reciprocal_approx_fast(linv, l_ps)
                at = attn_pool.tile([128, 512], BF16, tag="at", name="at")
                nc.vector.tensor_mul(at, o_ps, linv)
                qsub = qi % 4
                nc.gpsimd.dma_start(
                    out=ag_wr[nb][:, :, qsub * 128 : (qsub + 1) * 128], in_=at
                )

            # ---- seq-chunked AllGather (overlaps all remaining compute)
            nc.gpsimd.collective_compute(
                "AllGather",
                mybir.AluOpType.bypass,
                replica_groups=[list(range(NCORES))],
                ins=[ag_ins[nb][:]],
                outs=[ag_outs[nb][:]],
            )

        psA.release()
        hs_pool.release()

        # ---------- c_proj: y[mt] = attnT[:, mt].T @ wp_shard ----------
        with (
            tc.tile_pool(name="wp", bufs=1) as wp_pool,
            tc.tile_pool(name="lh", bufs=2) as lh_pool,
            tc.tile_pool(name="ysb", bufs=2) as y_pool,
            tc.tile_pool(name="psC", bufs=2, space="PSUM") as psC,
        ):
            wp_sb = wp_pool.tile([128, KC, P_SHARD], BF16)  # 32KB/part
            for kk in range(0, KC, 8):
                nc.sync.dma_start(
                    out=wp_sb[:, kk : kk + 8, :], in_=wp_v[:, kk : kk + 8, :]
                )
            for c in range(NB):
                for sub in range(4):
                    mt = c * 4 + sub
                    lh = lh_pool.tile([128, KC, 128], BF16, name="lh")
                    nc.sync.dma_start(
                        out=lh, in_=ag_rd[c][:, :, sub * 128 : (sub + 1) * 128]
                    )
                    yp = psC.tile([128, 512], F32, name="yp")
                    for k in range(KC):
                        nc.tensor.matmul(
                            yp,
                            lhsT=lh[:, k, :],
                            rhs=wp_sb[:, k, :],
                            start=(k == 0),
                            stop=(k == KC - 1),
                        )
                    ysb = y_pool.tile([128, P_SHARD], F32, name="ysb")
                    nc.scalar.activation(
                        out=ysb,
                        in_=yp,
                        func=mybir.ActivationFunctionType.Copy,
                    )
                    nc.gpsimd.dma_start(
                        out=y_out[mt * 128 : (mt + 1) * 128, :], in_=ysb
                    )

        for p in (
            psLO,
            psST,
            attn_pool,
            pt_pool,
            rope_pool,
            vnat_pool,
            const_pool,
            w_pool,
            qkv_pool,
        ):
            p.release()

    nc.compile()
    return nc


_CACHED = {}


def _get_module():
    if "nc" not in _CACHED:
        _CACHED["nc"] = build_module()
    return _CACHED["nc"]


def make_in_maps(hidden_states, w_attn, w_proj, rope_cos, rope_sin):
    hidden_states = np.asarray(hidden_states, dtype=np.float32)
    w_attn = np.asarray(w_attn, dtype=np.float32)
    w_proj = np.asarray(w_proj, dtype=np.float32)
    rope_cos = np.asarray(rope_cos, dtype=np.float32)
    rope_sin = np.asarray(rope_sin, dtype=np.float32)

    hs_t = np.ascontiguousarray(hidden_states.reshape(S, H).T).astype(BF16NP)
    cos_t = np.ascontiguousarray(rope_cos.T).astype(BF16NP)
    sin_t = np.ascontiguousarray(rope_sin.T).astype(BF16NP)

    # rotate-half as a matmul: rot(x) = R @ x for x in [HD, S] layout,
    # rot_t = R.T so that lhsT.T @ x = R @ x
    rot_t = np.zeros((HD, HD), dtype=np.float32)
    half = HD // 2
    rot_t[half + np.arange(half), np.arange(half)] = -1.0
    rot_t[np.arange(half), half + np.arange(half)] = 1.0
    rot_t = rot_t.astype(BF16NP)

    # additive causal mask for the diagonal 128x128 block, repeated for
    # the 4 fused heads: masks[k, h*128+qq] = MASKBIG iff qq < k
    kk_, qq_ = np.meshgrid(np.arange(128), np.arange(128), indexing="ij")
    m128 = np.where(qq_ < kk_, MASKBIG, 0.0).astype(np.float32)
    masks = np.tile(m128, (1, GROUP)).astype(BF16NP)

    ones = np.ones((128, 128), dtype=np.float32).astype(BF16NP)
    ident = np.eye(128, dtype=np.float32).astype(BF16NP)

    in_maps = []
    for i in range(NCORES):
        wa_sh = w_attn[i * M_SHARD : (i + 1) * M_SHARD, :]
        wp_sh = w_proj[i * P_SHARD : (i + 1) * P_SHARD, :]
        in_maps.append(
            {
                "hs_t": hs_t,
                "wa_t": np.ascontiguousarray(wa_sh.T).astype(BF16NP),
                "wp_t": np.ascontiguousarray(wp_sh.T).astype(BF16NP),
                "cos_t": cos_t,
                "sin_t": sin_t,
                "rot_t": rot_t,
                "masks_in": masks,
                "ones_in": ones,
                "ident_in": ident,
            }
        )
    return in_maps


def kernel(hidden_states, w_attn, w_proj, rope_cos, rope_sin, **_unused):
    nc = _get_module()
    in_maps = make_in_maps(hidden_states, w_attn, w_proj, rope_cos, rope_sin)
    res = run_bass_kernel_spmd(nc, in_maps, core_ids=list(range(NCORES)))

    out = np.empty((S, H), dtype=np.float32)
    for i in range(NCORES):
        out[:, i * P_SHARD : (i + 1) * P_SHARD] = res.results[i]["y"]
    return out.reshape(B, S, H)
